# revision 35
# baseline (speedup 1.0000x reference)
"""3-layer GAT on trn2, 8 NeuronCores (SPMD). v2

Strategy (v2):
- Nodes are permuted and dealt to 8 cores (snake order by in-degree);
  each core owns a contiguous SHARD (12544 rows) of table rows and the
  98 dst-blocks (128 dst nodes each) made from them.
- Per layer: each core transforms its shard (feat|el|er = h @ [W|Wl|Wr])
  writing [feat|el] rows to a quarter-split bounce and keeping er
  SBUF-resident; as each quarter-shard finishes, an AllGather of that
  quarter is kicked off (4 sub-AllGathers per layer, Shared outputs)
  building 4 region tables of 25088 rows each (int16-gatherable).
- Edge phase runs 4 passes (pass r reads region r only, so sub-AllGather
  r+1 overlaps pass r's compute). Per 2048-edge gather call: dma_gather
  of [feat|el] rows by src; edges inside each 128-slot chunk are sorted
  by dst_local so the transposed one-hot (OT) is a "staircase" built
  from two compares (no PE broadcast needed); OE is built edge-major
  from an iota compare; per chunk a small PE matmul (OT @ er_block)
  delivers er per edge; w = exp(leaky_relu(el+er)); V = [w*feat | w];
  main PE matmul OE^T @ V accumulates [dst, 132] in PSUM per block,
  added into a per-block SBUF accumulator across the 4 passes.
- When a block's last pass-3 chunk lands, its epilogue runs eagerly
  (divide by denominator, residual/ELU or head-mean) so next-layer
  transform can start early. bounce/table are double-buffered by layer
  parity to let layers overlap.
"""
import numpy as np

import concourse.bacc as bacc
import concourse.bass as bass
import concourse.mybir as mybir
import concourse.tile as tile
from concourse.bass_utils import run_bass_kernel_spmd

P = 128
NCORES = 8
HEADS = 4
F = 32
D = 128            # feature dim (all layers)
TW = 256           # table row elems fp16 (512B)
CALL_CH = 8        # chunks per gather call (1024 idxs; HW max)
fp16 = mybir.dt.float16
fp32 = mybir.dt.float32
AF = mybir.ActivationFunctionType
OP = mybir.AluOpType


# ----------------------------------------------------------------- host side
def _preprocess(src, dst, n_nodes):
    """Build node permutation, per-core schedules and index tiles."""
    E = src.shape[0]
    shard = int(np.ceil(n_nodes / (NCORES * P))) * P          # 12544
    Q = shard // 4                                             # 3136
    REG = NCORES * Q                                           # 25088
    deg = np.bincount(dst, minlength=n_nodes)
    order = np.argsort(-deg, kind="stable")                    # desc degree
    # snake deal to cores
    core_of_pos = np.tile(np.concatenate([np.arange(NCORES),
                                          np.arange(NCORES)[::-1]]),
                          (n_nodes + 2 * NCORES - 1) // (2 * NCORES))[:n_nodes]
    node_core = np.empty(n_nodes, np.int64)
    node_core[order] = core_of_pos
    # position within core (by deal order -> desc degree within core)
    pos_of_node = np.empty(n_nodes, np.int64)
    core_nodes = []
    for c in range(NCORES):
        nodes_c = order[node_core[order] == c]
        core_nodes.append(nodes_c)
        pos_of_node[nodes_c] = np.arange(len(nodes_c))
    gmax = shard // P                                          # 98 blocks
    # group edges: core/block/dloc of dst, quarter-region of src
    e_core = node_core[dst]
    e_blk = pos_of_node[dst] // P
    e_dloc = (pos_of_node[dst] % P).astype(np.int16)
    e_rng = pos_of_node[src] // Q                              # 0..3
    e_idx = (node_core[src] * Q + pos_of_node[src] % Q).astype(np.int16)

    counts = np.zeros((NCORES, gmax, 4), np.int64)
    np.add.at(counts, (e_core, e_blk, e_rng), 1)
    l = np.ceil(counts / P).astype(np.int64).max(axis=0)       # [g, 4]
    assert (l > 0).all()
    G = gmax
    chunks_r = l.sum(axis=0)                                   # per pass
    chmax = int(chunks_r.max())
    ncalls_r = [int(np.ceil(c / CALL_CH)) for c in chunks_r]
    callmax = max(ncalls_r)

    starts = np.zeros((G, 4), np.int64)
    starts[1:] = np.cumsum(l[:-1], axis=0)

    NI = CALL_CH * P                                           # idxs per call
    idx_tiles = np.zeros((NCORES, 4, callmax, P, NI // 16), np.int16)
    drow = np.full((NCORES, 4, 1, chmax * P), 240, np.float16)
    dcol = np.full((NCORES, 4, P, chmax), 240, np.float16)

    eo = np.lexsort((e_rng, e_blk, e_core))
    es, eb, er_, ei, ed = (x[eo] for x in (e_core, e_blk, e_rng, e_idx, e_dloc))
    grp = es * (gmax * 4) + eb * 4 + er_
    sort_count = np.bincount(grp, minlength=NCORES * gmax * 4)
    within = np.arange(len(eo)) - np.repeat(
        np.concatenate([[0], np.cumsum(sort_count)[:-1]]), sort_count)

    slot = starts[eb, er_] * P + within                        # edge slot in pass
    for c in range(NCORES):
        m = es == c
        s, r, iv, dv = slot[m], er_[m], ei[m], ed[m]
        for rr in range(4):
            mm = r == rr
            ss = s[mm]
            nch = int(chunks_r[rr])
            flat_i = np.zeros(nch * P, np.int16)
            flat_d = np.full(nch * P, 240, np.int16)
            flat_i[ss] = iv[mm]
            flat_d[ss] = dv[mm]
            drow[c, rr, 0, :nch * P] = flat_d.astype(np.float16)
            dcol[c, rr, :, :nch] = flat_d.reshape(nch, P).T.astype(np.float16)
            for k in range(ncalls_r[rr]):
                seg = flat_i[k * NI:(k + 1) * NI]
                n = len(seg)
                if n < NI:
                    seg = np.concatenate([seg, np.zeros(NI - n, np.int16)])
                idx_tiles[c, rr, k] = np.tile(seg.reshape(NI // 16, 16).T,
                                              (8, 1))

    sched = dict(shard=shard, Q=Q, REG=REG, G=G, l=l, chunks_r=chunks_r,
                 ncalls_r=ncalls_r, callmax=callmax, chmax=chmax,
                 starts=starts)
    data = dict(idx_tiles=idx_tiles, drow=drow, dcol=dcol)
    return core_nodes, sched, data


# ------------------------------------------------------------- device program
def _build(sched):
    shard, G, Q, REG = sched["shard"], sched["G"], sched["Q"], sched["REG"]
    l, chunks_r, ncalls_r = sched["l"], sched["chunks_r"], sched["ncalls_r"]
    callmax, chmax = sched["callmax"], sched["chmax"]
    GR = G * P
    NI = CALL_CH * P
    starts = sched["starts"]
    # quarter boundary rows within each transform tile
    qbound = {}                                  # tile -> split row (local)
    for qq in range(1, 4):
        t = (qq * Q) // P
        if qq * Q % P:
            qbound[t] = qq * Q % P

    nc = bacc.Bacc("TRN2", target_bir_lowering=False, debug=False,
                   num_devices=NCORES)
    feats = nc.dram_tensor("feats", [GR, D], fp16, kind="ExternalInput")
    wcat = nc.dram_tensor("wcat", [3, D, 136], fp16, kind="ExternalInput")
    idx_t = nc.dram_tensor("idx_t", [4, callmax, P, NI // 16], mybir.dt.int16,
                           kind="ExternalInput")
    drow_t = nc.dram_tensor("drow_t", [4, 1, chmax * P], fp16,
                            kind="ExternalInput")
    dcol_t = nc.dram_tensor("dcol_t", [4, P, chmax], fp16, kind="ExternalInput")
    out_sh = nc.dram_tensor("out_sh", [GR, F], fp32, kind="ExternalOutput")

    with tile.TileContext(nc) as tc:
        with (
            tc.tile_pool(name="const", bufs=1) as cp,
            tc.tile_pool(name="dram", bufs=1, space="DRAM") as dramp,
            tc.tile_pool(name="era", bufs=2) as erap,
            tc.tile_pool(name="io", bufs=12) as iop,
            tc.tile_pool(name="gath", bufs=6) as gp,
            tc.tile_pool(name="oh", bufs=8) as ohp,
            tc.tile_pool(name="sm", bufs=8) as smp,
            tc.tile_pool(name="acc", bufs=G) as accp,
            tc.tile_pool(name="slab", bufs=2) as slabp,
            tc.tile_pool(name="ps", bufs=3, space="PSUM") as psp,
            tc.tile_pool(name="pse", bufs=1, space="PSUM") as psep,
            tc.tile_pool(name="psb", bufs=2, space="PSUM") as psbp,
        ):
            # DRAM scratch (per-block h tiles for fine-grained deps)
            h1 = [dramp.tile([P, D], fp16, name=f"h1_{b}") for b in range(G)]
            h2 = [dramp.tile([P, D], fp16, name=f"h2_{b}") for b in range(G)]
            bq = [[dramp.tile([Q, TW], fp16, name=f"bq{p}_{q}")
                   for q in range(4)] for p in range(2)]
            tq = [[dramp.tile([REG, TW], fp16, name=f"tq{p}_{q}")
                   for q in range(4)] for p in range(3)]

            # constants
            chiota = cp.tile([P, 1], fp32)
            nc.gpsimd.iota(chiota[:], pattern=[[0, 1]], base=0,
                           channel_multiplier=1,
                           allow_small_or_imprecise_dtypes=True)
            iotaP = cp.tile([P, P], fp16)
            nc.gpsimd.iota(iotaP[:], pattern=[[1, P]], base=0,
                           channel_multiplier=0,
                           allow_small_or_imprecise_dtypes=True)
            iota8 = cp.tile([P, CALL_CH, P], fp16)
            nc.gpsimd.iota(iota8[:], pattern=[[0, CALL_CH], [1, P]], base=0,
                           channel_multiplier=0,
                           allow_small_or_imprecise_dtypes=True)
            ones_row = cp.tile([1, P], fp16)
            nc.vector.memset(ones_row[:], 1.0)
            wcs = []
            for L in range(3):
                wc = cp.tile([D, 136], fp16, tag="wc")
                nc.sync.dma_start(wc[:], wcat[L])
                wcs.append(wc)

            def hblk(L, b):
                if L == 0:
                    return feats[b * P:(b + 1) * P, :]
                return (h1 if L == 1 else h2)[b][:]

            for L in range(3):
                pi = L % 2
                er_all = erap.tile([P, G, 4], fp16, tag="er_all",
                                   name=f"era{L}")
                # ---- transform + quarter AllGathers
                for t in range(G):
                    hT = smp.tile([P, P], fp16, tag="hT")
                    nc.sync.dma_start(hT[:], hblk(L, t), transpose=True)
                    ptf = psp.tile([P, 136], fp32, space="PSUM", tag="pm")
                    nc.tensor.matmul(ptf[:], hT[:], wcs[L][:],
                                     start=True, stop=True)
                    stf = smp.tile([P, 136], fp16, tag="stf")
                    nc.scalar.activation(stf[:], ptf[:], AF.Copy)
                    nc.vector.tensor_copy(er_all[:, t, :], stf[:, 132:136])
                    r0 = t * P
                    qq = r0 // Q
                    if t in qbound:
                        sp = qbound[t]
                        nc.sync.dma_start(
                            bq[pi][qq][r0 - qq * Q:r0 - qq * Q + sp, 0:136],
                            stf[0:sp, 0:136])
                        nc.sync.dma_start(
                            bq[pi][qq + 1][0:P - sp, 0:136],
                            stf[sp:P, 0:136])
                    else:
                        nc.sync.dma_start(
                            bq[pi][qq][r0 - qq * Q:r0 - qq * Q + P, 0:136],
                            stf[:, 0:136])
                    if (t + 1) * P >= (r0 // Q + 1) * Q or t == G - 1:
                        # quarter qq rows complete -> allgather it
                        nc.gpsimd.collective_compute(
                            "AllGather", OP.bypass,
                            replica_groups=[list(range(NCORES))],
                            ins=[bq[pi][qq][:].opt()],
                            outs=[tq[L][qq][:].opt()])

                # per-block accumulators
                accs = [accp.tile([P, 132], fp32, tag="acc", name=f"acc{L}_{b}")
                        for b in range(G)]
                first_pass = [True] * G

                for r in range(4):
                    nch = int(chunks_r[r])
                    dcol_s = slabp.tile([P, chmax], fp16, tag="dcol")
                    nc.sync.dma_start(dcol_s[:, 0:nch], dcol_t[r, :, 0:nch])
                    blk_of = np.repeat(np.arange(G), l[:, r])
                    pm_tiles = {}
                    for k in range(ncalls_r[r]):
                        c0 = k * CALL_CH
                        ncc = min(CALL_CH, nch - c0)
                        ni = ncc * P
                        it = iop.tile([P, NI // 16], mybir.dt.int16, tag="idx")
                        nc.sync.dma_start(it[:, 0:ni // 16],
                                          idx_t[r, k, :, 0:ni // 16])
                        drow_c = iop.tile([1, NI], fp16, tag="drow")
                        nc.sync.dma_start(drow_c[:, 0:ni],
                                          drow_t[r, :, c0 * P:c0 * P + ni])
                        Gt = gp.tile([P, CALL_CH, TW], fp16, tag="G")
                        nc.gpsimd.dma_gather(
                            Gt[:, 0:ncc, :], tq[L][r][:],
                            it[:, 0:ni // 16], num_idxs=ni, num_idxs_reg=ni,
                            elem_size=TW)
                        # bcast dst_local row into psum
                        pbc = psbp.tile([P, NI], fp32, space="PSUM", tag="pbc")
                        for hh in range(0, ni, 512):
                            hw = min(512, ni - hh)
                            nc.tensor.matmul(
                                pbc[:, hh:hh + hw], ones_row[:],
                                drow_c[:, hh:hh + hw],
                                start=True, stop=True)
                        # OT dst-major one-hot (pbc+chiota fast path);
                        # OE edge-major via per-chunk [P,1]-scalar compare
                        OT = ohp.tile([P, CALL_CH, P], fp16, tag="OT")
                        nc.vector.tensor_scalar(
                            OT[:, 0:ncc, :],
                            pbc[:, 0:ni].rearrange("p (c e) -> p c e", e=P),
                            chiota[:], None, op0=OP.is_equal)
                        OE = ohp.tile([P, CALL_CH, P], fp16, tag="OE")
                        nc.vector.tensor_tensor(
                            out=OE[:, 0:ncc, :],
                            in0=iota8[:, 0:ncc, :],
                            in1=dcol_s[:, c0:c0 + ncc].unsqueeze(2)
                                .to_broadcast([P, ncc, P]),
                            op=OP.is_equal)
                        # er per edge
                        erp = psep.tile([P, CALL_CH * 4], fp32, space="PSUM",
                                        tag="er")
                        for c in range(ncc):
                            b = int(blk_of[c0 + c])
                            nc.tensor.matmul(erp[:, c * 4:(c + 1) * 4],
                                             OT[:, c, :], er_all[:, b, :],
                                             start=True, stop=True)
                        er16 = smp.tile([P, CALL_CH * 4], fp16, tag="er16")
                        nc.scalar.activation(er16[:, 0:ncc * 4],
                                             erp[:, 0:ncc * 4], AF.Copy)
                        e32 = smp.tile([P, CALL_CH * 4], fp32, tag="e32")
                        nc.vector.tensor_tensor(
                            out=e32[:, 0:ncc * 4]
                                .rearrange("p (c h) -> p c h", h=4),
                            in0=Gt[:, 0:ncc, 128:132],
                            in1=er16[:, 0:ncc * 4]
                                .rearrange("p (c h) -> p c h", h=4),
                            op=OP.add)
                        lr = smp.tile([P, CALL_CH * 4], fp32, tag="lr")
                        nc.vector.tensor_scalar_mul(lr[:, 0:ncc * 4],
                                                    e32[:, 0:ncc * 4], 0.2)
                        nc.vector.tensor_tensor(out=lr[:, 0:ncc * 4],
                                                in0=lr[:, 0:ncc * 4],
                                                in1=e32[:, 0:ncc * 4], op=OP.max)
                        w16 = smp.tile([P, CALL_CH * 4], fp16, tag="w16")
                        nc.scalar.activation(w16[:, 0:ncc * 4], lr[:, 0:ncc * 4],
                                             AF.Exp)
                        V = gp.tile([P, CALL_CH, 132], fp16, tag="V")
                        nc.vector.tensor_tensor(
                            out=V[:, 0:ncc, 0:128]
                                .rearrange("p c (h f) -> p c h f", f=F),
                            in0=Gt[:, 0:ncc, 0:128]
                                .rearrange("p c (h f) -> p c h f", f=F),
                            in1=w16[:, 0:ncc * 4]
                                .rearrange("p (c h) -> p c h", h=4)
                                .unsqueeze(3).to_broadcast([P, ncc, 4, F]),
                            op=OP.mult)
                        nc.scalar.activation(
                            V[:, 0:ncc, 128:132],
                            w16[:, 0:ncc * 4].rearrange("p (c h) -> p c h",
                                                        h=4),
                            AF.Copy)
                        # main matmuls, accumulate per block
                        for c in range(ncc):
                            gc = c0 + c
                            b = int(blk_of[gc])
                            sb_, lb = int(starts[b, r]), int(l[b, r])
                            if b not in pm_tiles:
                                pm_tiles[b] = psp.tile(
                                    [P, 136], fp32, space="PSUM", tag="pm",
                                    name=f"pm{L}_{r}_{b}")
                            nc.tensor.matmul(pm_tiles[b][:, 0:132],
                                             OE[:, c, :], V[:, c, :],
                                             start=(gc == sb_),
                                             stop=(gc == sb_ + lb - 1))
                            if gc == sb_ + lb - 1:
                                if first_pass[b]:
                                    nc.vector.tensor_copy(
                                        accs[b][:], pm_tiles[b][:, 0:132])
                                    first_pass[b] = False
                                else:
                                    nc.vector.tensor_tensor(
                                        out=accs[b][:], in0=accs[b][:],
                                        in1=pm_tiles[b][:, 0:132], op=OP.add)
                                del pm_tiles[b]
                                if r == 3:
                                    _epilogue(nc, smp, accs[b], hblk, out_sh,
                                              L, b)
    nc.compile()
    return nc


def _epilogue(nc, smp, acc, hblk, out_sh, L, b):
    rec = smp.tile([P, 4], fp32, tag="rec")
    nc.vector.reciprocal(rec[:], acc[:, 128:132])
    av = smp.tile([P, HEADS, F], fp32, tag="av")
    nc.vector.tensor_tensor(
        out=av[:],
        in0=acc[:, 0:128].rearrange("p (h f) -> p h f", f=F),
        in1=rec[:].unsqueeze(2).to_broadcast([P, HEADS, F]),
        op=OP.mult)
    if L < 2:
        if L >= 1:  # residual (L1 adds h1)
            hres = smp.tile([P, D], fp16, tag="hres")
            nc.sync.dma_start(hres[:], hblk(L, b))
            nc.vector.tensor_tensor(
                out=av[:], in0=av[:],
                in1=hres[:].rearrange("p (h f) -> p h f", f=F),
                op=OP.add)
        # ELU: relu(x) + exp(min(x,0)) - 1
        relu = smp.tile([P, D], fp32, tag="relu")
        nc.vector.tensor_scalar_max(
            relu[:], av[:].rearrange("p h f -> p (h f)"), 0.0)
        mn = smp.tile([P, D], fp32, tag="mn")
        nc.vector.tensor_scalar_min(
            mn[:], av[:].rearrange("p h f -> p (h f)"), 0.0)
        ex = smp.tile([P, D], fp32, tag="ex")
        nc.scalar.activation(ex[:], mn[:], AF.Exp)
        hnext = smp.tile([P, D], fp16, tag="hnext")
        nc.vector.tensor_tensor(out=ex[:], in0=ex[:], in1=relu[:], op=OP.add)
        nc.vector.tensor_scalar_add(hnext[:], ex[:], -1.0)
        nc.sync.dma_start(hblk(L + 1, b), hnext[:])
    else:
        # residual + mean over heads
        hres = smp.tile([P, D], fp16, tag="hres")
        nc.sync.dma_start(hres[:], hblk(2, b))
        nc.vector.tensor_tensor(
            out=av[:], in0=av[:],
            in1=hres[:].rearrange("p (h f) -> p h f", f=F),
            op=OP.add)
        o32 = smp.tile([P, F], fp32, tag="o32")
        nc.vector.tensor_tensor(out=o32[:], in0=av[:, 0, :],
                                in1=av[:, 1, :], op=OP.add)
        nc.vector.tensor_tensor(out=o32[:], in0=o32[:],
                                in1=av[:, 2, :], op=OP.add)
        nc.vector.tensor_tensor(out=o32[:], in0=o32[:],
                                in1=av[:, 3, :], op=OP.add)
        nc.vector.tensor_scalar_mul(o32[:], o32[:], 0.25)
        nc.sync.dma_start(out_sh[b * P:(b + 1) * P, :], o32[:])


_CACHE = {}
LAST_RESULTS = None


def kernel(**inputs):
    feats_f32 = np.asarray(inputs["features"], np.float32)
    src = np.asarray(inputs["src"]).astype(np.int64)
    dst = np.asarray(inputs["dst"]).astype(np.int64)
    n_nodes = feats_f32.shape[0]

    core_nodes, sched, data = _preprocess(src, dst, n_nodes)
    G = sched["G"]

    # weights: Wcat[L] = [W | Wl | Wr] with Wl = sum_f W[:,h,f]*al[h,f]
    wcat = np.zeros((3, D, 136), np.float16)
    for L, (wn, an, bn) in enumerate([("W0", "al0", "ar0"),
                                      ("W1", "al1", "ar1"),
                                      ("W2", "al2", "ar2")]):
        W = np.asarray(inputs[wn], np.float32)
        al = np.asarray(inputs[an], np.float32)
        ar = np.asarray(inputs[bn], np.float32)
        Wh = W.reshape(D, HEADS, F)
        wcat[L, :, 0:128] = W.astype(np.float16)
        wcat[L, :, 128:132] = np.einsum("dhf,hf->dh", Wh, al).astype(np.float16)
        wcat[L, :, 132:136] = np.einsum("dhf,hf->dh", Wh, ar).astype(np.float16)

    key = (n_nodes, src.shape[0])
    if key not in _CACHE:
        _CACHE[key] = _build(sched)
    nc = _CACHE[key]

    in_maps = []
    for c in range(NCORES):
        f16 = np.zeros((G * P, D), np.float16)
        nodes_c = core_nodes[c]
        f16[:len(nodes_c)] = feats_f32[nodes_c].astype(np.float16)
        in_maps.append({
            "feats": f16,
            "wcat": wcat,
            "idx_t": data["idx_tiles"][c],
            "drow_t": data["drow"][c],
            "dcol_t": data["dcol"][c],
        })

    import os
    trace = bool(int(os.environ.get("TRN_KERNEL_TRACE", "0")))
    res = run_bass_kernel_spmd(nc, in_maps, core_ids=list(range(NCORES)),
                               trace=trace)
    global LAST_RESULTS
    LAST_RESULTS = res
    out = np.zeros((n_nodes, F), np.float32)
    for c in range(NCORES):
        nodes_c = core_nodes[c]
        out[nodes_c] = res.results[c]["out_sh"][:len(nodes_c)]
    return out
